# revision 29
# baseline (speedup 1.0000x reference)
"""Trainium2 Bass kernel for 16-head self-attention (b=2, n=2048, dm=1024, dh=64).

Sharding: (batch x head-quad).  Core c owns batch g = c//4 and heads
[4*(c%4) .. 4*(c%4)+3], a 256-column slice of the inner dimension.  Every
projection is computed exactly once across the chip (393k PE cycles/core vs
590k for the replicated batch x seq sharding of v1).  Each core's output
projection is a PARTIAL sum over its 256 inner dims; the host sums the four
partials per batch during the unshard (the "all-reduce after to_out" of the
sharding hint, folded into the host gather).

All matmuls are bf16 (separate LDWEIGHTS instructions hide behind in-flight
matmuls via the PE reorder window; f32r self-loads weights serially and
measured +34us/core).  Everything is SBUF-resident.

S^T is ROW-TILED: each head contracts only dh=64, so the two heads of a
pair run CONCURRENTLY in disjoint 64-row strips of the PE array
(tile_position auto-derived from base partition 0 / 64) -- one ~512-cycle
pass produces S^T for both heads, halving S matmul time vs the v3
zero-padded-contraction scheme and making Q^T zero-padding unnecessary.

Layouts (no on-chip transposes):
  Q^T[i,q] = (Wq slice as lhsT) @ (x^T as rhs)
  K^T[i,k] = (Wk slice as lhsT) @ (x^T as rhs)
  V [k,i]  = (x^T as lhsT) @ (Wv slice as rhs)  stored as [V_h | 1 | 0pad]
             so PSUM row 64 of the O matmul is the softmax denominator
  S^T[k,q] = (K^T rows of head h as lhsT) @ (Q^T rows of head h as rhs),
             both heads concurrent in row strips -> [128, 2, 512] PSUM
  O''[d,q] = ([V|1|0] as lhsT) @ (exp S^T as rhs)  accumulated over 16 kb
  out[q,d] = (O^T as lhsT) @ (Wo slice as rhs)   partial; host sums

exp runs on ACT over [128,1024] two-bank PSUM groups (one per key block,
both heads) -- 128 instructions at (1024+352)/1.2 ns = 143us/core, the
attention pacing engine.  The PE stream interleaves projection /
output-projection work into the ACT-bound idle via a deadline-scheduled
filler list (projection bundles split into 4-contraction-step halves so a
single pop never stalls the exp pipe by more than ~0.9us).  PSUM: S 2x2 +
O 2 + proj 2 = 8 banks.

Softmax finalize is early-release: the denominator row and the 64
numerator rows are staged out of PSUM with two DVE copies (releasing the
O PSUM bank for the next query chunk immediately); reciprocal_approx_fast
(on the SBUF copy -- the custom DVE op misreads PSUM operands) + GPSIMD
partition_broadcast + multiply then run off the critical path.
"""

import sys

for _p in ("/opt/trn_rl_repo", "/root/.axon_site/_ro/trn_rl_repo"):
    if _p not in sys.path:
        sys.path.append(_p)

import numpy as np

B = 2
N = 2048
DM = 1024
H = 16
DH = 64
INNER = H * DH  # 1024
NCORES = 8
HS = 256        # inner slice per core (4 heads)
SCALE = DH ** -0.5

A = DM // 128   # 8 dm blocks
KB = N // 128   # 16 key blocks
QC = N // 512   # 4 query chunks

_cached = {}


def _build():
    import contextlib
    import concourse.bacc as bacc
    import concourse.tile as tile
    import concourse.mybir as mybir

    f32 = mybir.dt.float32
    bf16 = mybir.dt.bfloat16
    Exp = mybir.ActivationFunctionType.Exp

    nc = bacc.Bacc("TRN2", target_bir_lowering=False, debug=False,
                   enable_asserts=False)

    xT_d = nc.dram_tensor("xT", [DM, N], bf16, kind="ExternalInput").ap()
    Wq_d = nc.dram_tensor("Wq", [DM, HS], bf16, kind="ExternalInput").ap()
    Wk_d = nc.dram_tensor("Wk", [DM, HS], bf16, kind="ExternalInput").ap()
    Wv_d = nc.dram_tensor("Wv", [DM, HS], bf16, kind="ExternalInput").ap()
    Wo_d = nc.dram_tensor("Wo", [HS, DM], bf16, kind="ExternalInput").ap()
    out_d = nc.dram_tensor("out", [N, DM], f32, kind="ExternalOutput").ap()

    with tile.TileContext(nc) as tc, \
         nc.allow_low_precision(reason="bf16 matmul pipeline, validated "
                                       "e2e vs f32 reference"), \
         contextlib.ExitStack() as ctx:
        persist = ctx.enter_context(tc.tile_pool(name="persist", bufs=1))
        QT_nat = persist.tile([128, 2, N], bf16)       # [pair dims, pair, q]
        KT = persist.tile([128, 2, N], bf16)           # [pair dims, pair, k]
        V_aug = persist.tile([128, 4, KB, 128], bf16)  # [keys, head, kb, V|1|0]
        OT = persist.tile([128, 2, N], bf16)           # [pair dims, pair, q]
        Wo_sb = persist.tile([128, 2, DM], bf16)
        ozpat = persist.tile([128, 64], f32)           # col0=1, cols1..63=0

        pa_x = ctx.enter_context(tc.tile_pool(name="pa_x", bufs=1))
        pa_w = ctx.enter_context(tc.tile_pool(name="pa_w", bufs=1))
        pes = ctx.enter_context(tc.tile_pool(name="pes", bufs=8))
        pv = ctx.enter_context(tc.tile_pool(name="pv", bufs=2))
        pstg = ctx.enter_context(tc.tile_pool(name="pstg", bufs=4))
        ps_p = ctx.enter_context(
            tc.tile_pool(name="ps_p", bufs=2, space="PSUM"))
        psS = ctx.enter_context(
            tc.tile_pool(name="psS", bufs=2, space="PSUM"))
        psO = ctx.enter_context(
            tc.tile_pool(name="psO", bufs=2, space="PSUM"))

        xT_sb = pa_x.tile([128, A, N], bf16)
        Wq_sb = pa_w.tile([128, A, HS], bf16)
        Wk_sb = pa_w.tile([128, A, HS], bf16)
        Wv_sb = pa_w.tile([128, A, HS], bf16)

        # constant pads: DVE broadcast fill during the DMA-gated dead time
        nc.vector.memset(ozpat, 0.0)
        nc.vector.memset(ozpat[:, 0:1], 1.0)
        nc.vector.tensor_copy(
            out=V_aug[:, :, :, 64:128],
            in_=ozpat.unsqueeze(1).unsqueeze(1).to_broadcast(
                [128, 4, KB, 64]))

        # PE warm-up: ~4us of junk matmuls during the DMA-gated dead time
        # release the HAM clock throttle (4/8 -> 8/8) before real work
        wu = persist.tile([64, 64], bf16)
        nc.vector.tensor_copy(out=wu, in_=ozpat[0:64, :])
        wup = ps_p.tile([64, 64], f32, tag="qk", name="wup")
        for _ in range(40):
            nc.tensor.matmul(out=wup, lhsT=wu, rhs=wu, start=True,
                             stop=True)

        xT_r = xT_d.rearrange("(a p) n -> a p n", p=128)
        # input loads split across the sync and vector DMA rings so the
        # start gate halves; within each ring, what the PE needs first,
        # first.  (NOT the gpsimd ring: pool-ring input loads alongside
        # partition_broadcast wedged the exec unit.  The ACT-ring posts
        # all happen before the first exp, so they cost ACT nothing.)
        nc.sync.dma_start(out=Wq_sb,
                          in_=Wq_d.rearrange("(a p) i -> p a i", p=128))
        nc.scalar.dma_start(out=Wv_sb,
                            in_=Wv_d.rearrange("(a p) i -> p a i", p=128))
        for a in range(A):
            nc.sync.dma_start(out=xT_sb[:, a, 0:512], in_=xT_r[a][:, 0:512])
            nc.scalar.dma_start(out=xT_sb[:, a, 512:1024],
                                in_=xT_r[a][:, 512:1024])
        nc.sync.dma_start(out=Wk_sb,
                          in_=Wk_d.rearrange("(a p) i -> p a i", p=128))
        for a in range(A):
            nc.sync.dma_start(out=xT_sb[:, a, 1024:1536],
                              in_=xT_r[a][:, 1024:1536])
            nc.scalar.dma_start(out=xT_sb[:, a, 1536:2048],
                                in_=xT_r[a][:, 1536:2048])
        nc.sync.dma_start(out=Wo_sb,
                          in_=Wo_d.rearrange("(ib p) d -> p ib d", p=128))

        # ---- emission helpers ----
        def emit_qt(p, qc, state=None, half=None):
            if state is None:
                state = {}
                halves = (0, 1)
            else:
                halves = (half,)
            for hf in halves:
                if hf == 0:
                    state['t'] = ps_p.tile([128, 512], f32, tag="qk",
                                           name="qp")
                qp = state['t']
                for a in range(hf * 4, hf * 4 + 4):
                    nc.tensor.matmul(
                        out=qp,
                        lhsT=Wq_sb[:, a, p * 128:(p + 1) * 128],
                        rhs=xT_sb[:, a, qc * 512:(qc + 1) * 512],
                        start=(a == 0), stop=(a == A - 1))
                if hf == 1:
                    nc.vector.tensor_copy(
                        out=QT_nat[:, p, qc * 512:(qc + 1) * 512], in_=qp)

        def emit_kt(p, kc, state=None, half=None):
            if state is None:
                state = {}
                halves = (0, 1)
            else:
                halves = (half,)
            for hf in halves:
                if hf == 0:
                    state['t'] = ps_p.tile([128, 512], f32, tag="qk",
                                           name="kp")
                kp = state['t']
                for a in range(hf * 4, hf * 4 + 4):
                    nc.tensor.matmul(
                        out=kp,
                        lhsT=Wk_sb[:, a, p * 128:(p + 1) * 128],
                        rhs=xT_sb[:, a, kc * 512:(kc + 1) * 512],
                        start=(a == 0), stop=(a == A - 1))
                if hf == 1:
                    nc.vector.tensor_copy(
                        out=KT[:, p, kc * 512:(kc + 1) * 512], in_=kp)

        def proj_halves(fn, p, i):
            st = {}
            return [lambda: fn(p, i, st, 0), lambda: fn(p, i, st, 1)]

        def emit_v(kb, state=None, half=None):
            if state is None:
                state = {}
                halves = (0, 1)
            else:
                halves = (half,)
            for hf in halves:
                if hf == 0:
                    state['t'] = ps_p.tile([128, HS], f32, tag="qk",
                                           name="vp")
                vp = state['t']
                for a in range(hf * 4, hf * 4 + 4):
                    nc.tensor.matmul(
                        out=vp,
                        lhsT=xT_sb[:, a, kb * 128:(kb + 1) * 128],
                        rhs=Wv_sb[:, a, :],
                        start=(a == 0), stop=(a == A - 1))
                if hf == 1:
                    nc.vector.tensor_copy(
                        out=V_aug[:, :, kb, 0:64],
                        in_=vp.rearrange("p (h d) -> p h d", h=4))

        def v_halves(kb):
            st = {}
            return [lambda: emit_v(kb, st, 0), lambda: emit_v(kb, st, 1)]

        def emit_outproj(qb, dc, last=False):
            outp = ps_p.tile([128, 512], f32, tag="qk", name="outp")
            for p in range(2):
                nc.tensor.matmul(
                    out=outp,
                    lhsT=OT[:, p, qb * 128:(qb + 1) * 128],
                    rhs=Wo_sb[:, p, dc * 512:(dc + 1) * 512],
                    start=(p == 0), stop=(p == 1))
            ob = pstg.tile([128, 512], f32, tag="ob", name="ob")
            if last:
                # tail: spread drain copies over ACT + DVE, DMAs over rings
                if (qb + dc) % 2:
                    nc.scalar.copy(out=ob, in_=outp)
                    dma = nc.gpsimd.dma_start
                else:
                    nc.vector.tensor_copy(out=ob, in_=outp)
                    dma = nc.sync.dma_start
                dma(out=out_d[qb * 128:(qb + 1) * 128,
                              dc * 512:(dc + 1) * 512],
                    in_=ob)
            else:
                nc.vector.tensor_copy(out=ob, in_=outp)
                nc.sync.dma_start(
                    out=out_d[qb * 128:(qb + 1) * 128,
                              dc * 512:(dc + 1) * 512],
                    in_=ob)

        # ---- software-pipelined O: S/exp run LAG key-blocks ahead of the
        # O accumulation, so projection fillers get LAG grps of deadline
        # slack and the PE never stalls on an exp it just requested ----
        LAG = 6
        ostate = {"q": [], "grp": 0, "ops": {}}

        def finalize(p, qc):
            ops = ostate["ops"].pop((p, qc))
            for hh in range(2):
                # early release: stage denominator + numerator out of PSUM,
                # then normalize off the critical path
                den = pv.tile([1, 512], f32, tag="den", name="den")
                nc.vector.tensor_copy(out=den, in_=ops[hh][64:65, :])
                onum = pv.tile([64, 512], f32, tag="onum", name="onum")
                nc.vector.tensor_copy(out=onum, in_=ops[hh][0:64, :])
                recip = pv.tile([1, 512], f32, tag="recip", name="recip")
                nc.vector.reciprocal_approx_fast(out=recip, in_=den)
                rbs = pv.tile([64, 512], f32, tag="rbs", name="rbs")
                nc.gpsimd.partition_broadcast(rbs, recip)
                nc.vector.tensor_mul(
                    OT[hh * 64:(hh + 1) * 64, p, qc * 512:(qc + 1) * 512],
                    onum, rbs)

        def drain_one():
            p, qc, kb, es = ostate["q"].pop(0)
            if kb == 0:
                ostate["ops"][(p, qc)] = [
                    psO.tile([128, 512], f32, tag="o", name=f"op{hh}")
                    for hh in range(2)]
            ops = ostate["ops"][(p, qc)]
            for hh in range(2):
                nc.tensor.matmul(
                    out=ops[hh],
                    lhsT=V_aug[:, p * 2 + hh, kb, :],
                    rhs=es[:, hh, :],
                    start=(kb == 0), stop=(kb == KB - 1))
            if kb == KB - 1:
                finalize(p, qc)

        def attn_qc(p, qc, st, last=False):
            for kb in range(KB):
                npop = st[0].pop(0) if st[0] else 0
                for _ in range(npop):
                    if st[1]:
                        st[1].pop(0)()
                sp = psS.tile([128, 2, 512], f32, tag="s", name="sp")
                for hh in range(2):
                    nc.tensor.matmul(
                        out=sp[:, hh, :],
                        lhsT=KT[hh * 64:(hh + 1) * 64, p,
                                kb * 128:(kb + 1) * 128],
                        rhs=QT_nat[hh * 64:(hh + 1) * 64, p,
                                   qc * 512:(qc + 1) * 512],
                        start=True, stop=True)
                es = pes.tile([128, 2, 512], bf16, tag="es", name="es")
                nc.scalar.activation(out=es, in_=sp, func=Exp, scale=SCALE)
                ostate["q"].append((p, qc, kb, es))
                ostate["grp"] += 1
                if ostate["grp"] > LAG:
                    drain_one()
                if last and kb >= 10 and ostate["q"]:
                    drain_one()  # empty the pipeline with the attention

        # ---------------- emission order (= PE stream order) -------------
        emit_qt(0, 0)
        emit_kt(0, 0)
        emit_qt(0, 1)
        emit_v(0)
        emit_v(1)

        OP = lambda qb, dc: (lambda: emit_outproj(qb, dc))

        # deadline-checked filler schedule (LAG=6 deferral): V(kb) must pop
        # by grp kb+LAG (where its first O matmul drains); K^T chunk kc by
        # the S matmul of grp 4*kc (pops precede the S matmuls of the same
        # grp); pair-1 projections by the end of (0,3).  V bundles split in
        # 4-contraction-step halves (0.44us) so early grps stay under the
        # 1.15us exp pace.
        sched = {
            (0, 0): ([2, 2, 2, 2, 2, 2] + [1] * 10,
                     proj_halves(emit_kt, 0, 1)
                     + v_halves(2) + v_halves(3) + v_halves(4) + v_halves(5)
                     + proj_halves(emit_kt, 0, 2)
                     + v_halves(6)
                     + proj_halves(emit_kt, 0, 3)
                     + v_halves(7) + v_halves(8) + v_halves(9)),
            (0, 1): ([2, 2, 2, 2, 2, 2] + [1] * 4 + [0] * 6,
                     v_halves(10) + v_halves(11) + v_halves(12)
                     + v_halves(13) + v_halves(14) + v_halves(15)
                     + proj_halves(emit_qt, 0, 2)
                     + proj_halves(emit_qt, 0, 3)),
            (0, 2): ([1] * 10 + [0] * 6,
                     proj_halves(emit_qt, 1, 0)
                     + proj_halves(emit_kt, 1, 0)
                     + proj_halves(emit_qt, 1, 1)
                     + proj_halves(emit_kt, 1, 1)
                     + proj_halves(emit_qt, 1, 2)),
            (0, 3): ([1] * 6 + [0] * 10,
                     proj_halves(emit_kt, 1, 2)
                     + proj_halves(emit_qt, 1, 3)
                     + proj_halves(emit_kt, 1, 3)),
            (1, 0): ([0] * 16, []),
            (1, 1): ([0] * 8 + [1] * 8,
                     [OP(qb, dc) for qb in range(0, 4) for dc in range(2)]),
            (1, 2): ([0] * 8 + [1] * 8,
                     [OP(qb, dc) for qb in range(4, 8) for dc in range(2)]),
            (1, 3): ([0] * 6 + [1] * 8 + [0] * 2,
                     [OP(qb, dc) for qb in range(8, 12) for dc in range(2)]),
        }
        carry = []
        for p in range(2):
            for qc in range(QC):
                pops, fills = sched[(p, qc)]
                st = [list(pops), carry + fills]
                attn_qc(p, qc, st, last=(p == 1 and qc == 3))
                carry = st[1]
        for f in carry:
            f()
        while ostate["q"]:
            drain_one()
        for qb in range(12, 16):
            for dc in range(2):
                emit_outproj(qb, dc, last=True)

    nc.compile()
    return nc


def _get_nc():
    if "nc" not in _cached:
        _cached["nc"] = _build()
    return _cached["nc"]


def kernel(queries, Wq, Wkv, Wo, bo, _trace=False):
    import ml_dtypes
    from concourse.bass_utils import run_bass_kernel_spmd

    bf = ml_dtypes.bfloat16
    queries = np.asarray(queries, dtype=np.float32)
    Wq = np.asarray(Wq, dtype=np.float32)
    Wkv = np.asarray(Wkv, dtype=np.float32)
    Wo = np.asarray(Wo, dtype=np.float32)
    bo = np.asarray(bo, dtype=np.float32)

    nc = _get_nc()

    xT_g = [np.ascontiguousarray(queries[g].T).astype(bf) for g in range(B)]
    in_maps = []
    for c in range(NCORES):
        g, hq = c // 4, c % 4
        s, e = hq * HS, (hq + 1) * HS
        in_maps.append({
            "xT": xT_g[g],
            "Wq": np.ascontiguousarray(Wq[:, s:e]).astype(bf),
            "Wk": np.ascontiguousarray(Wkv[:, s:e]).astype(bf),
            "Wv": np.ascontiguousarray(Wkv[:, INNER + s:INNER + e]).astype(bf),
            "Wo": np.ascontiguousarray(Wo[s:e, :]).astype(bf),
        })

    res = run_bass_kernel_spmd(nc, in_maps, list(range(NCORES)),
                               trace=_trace)
    out = np.empty((B, N, DM), dtype=np.float32)
    for g in range(B):
        acc = res.results[4 * g]["out"].astype(np.float32)
        for r in range(1, 4):
            acc = acc + res.results[4 * g + r]["out"]
        out[g] = acc + bo[None, :]
    if _trace:
        return out, res
    return out


if __name__ == "__main__":
    rng = np.random.default_rng(0)
    s = 0.02
    inputs = dict(
        queries=rng.standard_normal((B, N, DM), dtype=np.float32),
        Wq=(rng.standard_normal((DM, INNER), dtype=np.float32) * s),
        Wkv=(rng.standard_normal((DM, 2 * INNER), dtype=np.float32) * s),
        Wo=(rng.standard_normal((INNER, DM), dtype=np.float32) * s),
        bo=(rng.standard_normal((DM,), dtype=np.float32) * s),
    )
    out = kernel(**inputs)
    print("kernel ran, out shape", out.shape)

    q = inputs["queries"]
    qp = q @ inputs["Wq"]
    kv = q @ inputs["Wkv"]
    k, v = np.split(kv, 2, axis=-1)
    sh = lambda t: t.reshape(B, N, H, DH).transpose(0, 2, 1, 3)
    qp, k, v = map(sh, (qp, k, v))
    sim = np.einsum('bhid,bhjd->bhij', qp, k) * SCALE
    sim = np.exp(sim - sim.max(-1, keepdims=True))
    attn = sim / sim.sum(-1, keepdims=True)
    o = np.einsum('bhij,bhjd->bhid', attn, v)
    o = o.transpose(0, 2, 1, 3).reshape(B, N, INNER)
    exp = o @ inputs["Wo"] + inputs["bo"]
    err = np.linalg.norm((out - exp).ravel()) / np.linalg.norm(exp.ravel())
    print("numpy rel err:", err)


# revision 31
# speedup vs baseline: 1.0274x; 1.0274x over previous
"""Trainium2 Bass kernel for 16-head self-attention (b=2, n=2048, dm=1024, dh=64).

Sharding: (batch x head-quad).  Core c owns batch g = c//4 and heads
[4*(c%4) .. 4*(c%4)+3], a 256-column slice of the inner dimension.  Every
projection is computed exactly once across the chip (393k PE cycles/core vs
590k for the replicated batch x seq sharding of v1).  Each core's output
projection is a PARTIAL sum over its 256 inner dims; the host sums the four
partials per batch during the unshard (the "all-reduce after to_out" of the
sharding hint, folded into the host gather).

All matmuls are bf16 (separate LDWEIGHTS instructions hide behind in-flight
matmuls via the PE reorder window; f32r self-loads weights serially and
measured +34us/core).  Everything is SBUF-resident.

S^T is ROW-TILED: each head contracts only dh=64, so the two heads of a
pair run CONCURRENTLY in disjoint 64-row strips of the PE array
(tile_position auto-derived from base partition 0 / 64) -- one ~512-cycle
pass produces S^T for both heads, halving S matmul time vs the v3
zero-padded-contraction scheme and making Q^T zero-padding unnecessary.

Layouts (no on-chip transposes):
  Q^T[i,q] = (Wq slice as lhsT) @ (x^T as rhs)
  K^T[i,k] = (Wk slice as lhsT) @ (x^T as rhs)
  V [k,i]  = (x^T as lhsT) @ (Wv slice as rhs)  stored as [V_h | 1 | 0pad]
             so PSUM row 64 of the O matmul is the softmax denominator
  S^T[k,q] = (K^T rows of head h as lhsT) @ (Q^T rows of head h as rhs),
             both heads concurrent in row strips -> [128, 2, 512] PSUM
  O''[d,q] = ([V|1|0] as lhsT) @ (exp S^T as rhs)  accumulated over 16 kb
  out[q,d] = (O^T as lhsT) @ (Wo slice as rhs)   partial; host sums

exp runs on ACT over [128,1024] two-bank PSUM groups (one per key block,
both heads) -- 128 instructions at (1024+352)/1.2 ns = 143us/core, the
attention pacing engine.  The PE stream interleaves projection /
output-projection work into the ACT-bound idle via a deadline-scheduled
filler list (projection bundles split into 4-contraction-step halves so a
single pop never stalls the exp pipe by more than ~0.9us).  PSUM: S 2x2 +
O 2 + proj 2 = 8 banks.

Softmax finalize is early-release: the denominator row and the 64
numerator rows are staged out of PSUM with two DVE copies (releasing the
O PSUM bank for the next query chunk immediately); reciprocal_approx_fast
(on the SBUF copy -- the custom DVE op misreads PSUM operands) + GPSIMD
partition_broadcast + multiply then run off the critical path.
"""

import sys

for _p in ("/opt/trn_rl_repo", "/root/.axon_site/_ro/trn_rl_repo"):
    if _p not in sys.path:
        sys.path.append(_p)

import numpy as np

B = 2
N = 2048
DM = 1024
H = 16
DH = 64
INNER = H * DH  # 1024
NCORES = 8
HS = 256        # inner slice per core (4 heads)
SCALE = DH ** -0.5

A = DM // 128   # 8 dm blocks
KB = N // 128   # 16 key blocks
QC = N // 512   # 4 query chunks

_cached = {}


def _build():
    import contextlib
    import concourse.bacc as bacc
    import concourse.tile as tile
    import concourse.mybir as mybir

    f32 = mybir.dt.float32
    bf16 = mybir.dt.bfloat16
    Exp = mybir.ActivationFunctionType.Exp

    nc = bacc.Bacc("TRN2", target_bir_lowering=False, debug=False,
                   enable_asserts=False)

    xT_d = nc.dram_tensor("xT", [DM, N], bf16, kind="ExternalInput").ap()
    Wq_d = nc.dram_tensor("Wq", [DM, HS], bf16, kind="ExternalInput").ap()
    Wk_d = nc.dram_tensor("Wk", [DM, HS], bf16, kind="ExternalInput").ap()
    Wv_d = nc.dram_tensor("Wv", [DM, HS], bf16, kind="ExternalInput").ap()
    Wo_d = nc.dram_tensor("Wo", [HS, DM], bf16, kind="ExternalInput").ap()
    out_d = nc.dram_tensor("out", [N, DM], f32, kind="ExternalOutput").ap()

    with tile.TileContext(nc) as tc, \
         nc.allow_low_precision(reason="bf16 matmul pipeline, validated "
                                       "e2e vs f32 reference"), \
         contextlib.ExitStack() as ctx:
        persist = ctx.enter_context(tc.tile_pool(name="persist", bufs=1))
        QT_nat = persist.tile([128, 2, N], bf16)       # [pair dims, pair, q]
        KT = persist.tile([128, 2, N], bf16)           # [pair dims, pair, k]
        V_aug = persist.tile([128, 4, KB, 128], bf16)  # [keys, head, kb, V|1|0]
        OT = persist.tile([128, 2, N], bf16)           # [pair dims, pair, q]
        Wo_sb = persist.tile([128, 2, DM], bf16)
        ozpat = persist.tile([128, 64], f32)           # col0=1, cols1..63=0

        pa_x = ctx.enter_context(tc.tile_pool(name="pa_x", bufs=1))
        pa_w = ctx.enter_context(tc.tile_pool(name="pa_w", bufs=1))
        pes = ctx.enter_context(tc.tile_pool(name="pes", bufs=8))
        pv = ctx.enter_context(tc.tile_pool(name="pv", bufs=2))
        pstg = ctx.enter_context(tc.tile_pool(name="pstg", bufs=4))
        ps_p = ctx.enter_context(
            tc.tile_pool(name="ps_p", bufs=2, space="PSUM"))
        psS = ctx.enter_context(
            tc.tile_pool(name="psS", bufs=2, space="PSUM"))
        psO = ctx.enter_context(
            tc.tile_pool(name="psO", bufs=2, space="PSUM"))

        xT_sb = pa_x.tile([128, A, N], bf16)
        Wq_sb = pa_w.tile([128, A, HS], bf16)
        Wk_sb = pa_w.tile([128, A, HS], bf16)
        Wv_sb = pa_w.tile([128, A, HS], bf16)

        # constant pads: DVE broadcast fill during the DMA-gated dead time
        nc.vector.memset(ozpat, 0.0)
        nc.vector.memset(ozpat[:, 0:1], 1.0)
        nc.vector.tensor_copy(
            out=V_aug[:, :, :, 64:128],
            in_=ozpat.unsqueeze(1).unsqueeze(1).to_broadcast(
                [128, 4, KB, 64]))

        xT_r = xT_d.rearrange("(a p) n -> a p n", p=128)
        # sync-ring DMA order: what the PE needs first, first.  (Loads on
        # the gpsimd ring alongside partition_broadcast wedge the exec
        # unit; ACT-ring posts cost ~0.4us of ACT each.)
        nc.sync.dma_start(out=Wq_sb,
                          in_=Wq_d.rearrange("(a p) i -> p a i", p=128))
        for a in range(A):
            nc.sync.dma_start(out=xT_sb[:, a, 0:512], in_=xT_r[a][:, 0:512])
        nc.sync.dma_start(out=Wk_sb,
                          in_=Wk_d.rearrange("(a p) i -> p a i", p=128))
        for a in range(A):
            nc.sync.dma_start(out=xT_sb[:, a, 512:1024],
                              in_=xT_r[a][:, 512:1024])
        nc.sync.dma_start(out=Wv_sb,
                          in_=Wv_d.rearrange("(a p) i -> p a i", p=128))
        for qc in range(2, QC):
            for a in range(A):
                nc.sync.dma_start(out=xT_sb[:, a, qc * 512:(qc + 1) * 512],
                                  in_=xT_r[a][:, qc * 512:(qc + 1) * 512])
        nc.sync.dma_start(out=Wo_sb,
                          in_=Wo_d.rearrange("(ib p) d -> p ib d", p=128))

        # ---- emission helpers ----
        def emit_qt(p, qc, state=None, half=None):
            if state is None:
                state = {}
                halves = (0, 1)
            else:
                halves = (half,)
            for hf in halves:
                if hf == 0:
                    state['t'] = ps_p.tile([128, 512], f32, tag="qk",
                                           name="qp")
                qp = state['t']
                for a in range(hf * 4, hf * 4 + 4):
                    nc.tensor.matmul(
                        out=qp,
                        lhsT=Wq_sb[:, a, p * 128:(p + 1) * 128],
                        rhs=xT_sb[:, a, qc * 512:(qc + 1) * 512],
                        start=(a == 0), stop=(a == A - 1))
                if hf == 1:
                    nc.vector.tensor_copy(
                        out=QT_nat[:, p, qc * 512:(qc + 1) * 512], in_=qp)

        def emit_kt(p, kc, state=None, half=None):
            if state is None:
                state = {}
                halves = (0, 1)
            else:
                halves = (half,)
            for hf in halves:
                if hf == 0:
                    state['t'] = ps_p.tile([128, 512], f32, tag="qk",
                                           name="kp")
                kp = state['t']
                for a in range(hf * 4, hf * 4 + 4):
                    nc.tensor.matmul(
                        out=kp,
                        lhsT=Wk_sb[:, a, p * 128:(p + 1) * 128],
                        rhs=xT_sb[:, a, kc * 512:(kc + 1) * 512],
                        start=(a == 0), stop=(a == A - 1))
                if hf == 1:
                    nc.vector.tensor_copy(
                        out=KT[:, p, kc * 512:(kc + 1) * 512], in_=kp)

        def proj_halves(fn, p, i):
            st = {}
            return [lambda: fn(p, i, st, 0), lambda: fn(p, i, st, 1)]

        def emit_v(kb, state=None, half=None):
            if state is None:
                state = {}
                halves = (0, 1)
            else:
                halves = (half,)
            for hf in halves:
                if hf == 0:
                    state['t'] = ps_p.tile([128, HS], f32, tag="qk",
                                           name="vp")
                vp = state['t']
                for a in range(hf * 4, hf * 4 + 4):
                    nc.tensor.matmul(
                        out=vp,
                        lhsT=xT_sb[:, a, kb * 128:(kb + 1) * 128],
                        rhs=Wv_sb[:, a, :],
                        start=(a == 0), stop=(a == A - 1))
                if hf == 1:
                    # per-head copies: one strided all-head copy has a
                    # bounding box spanning the tile and serializes every
                    # later O matmul behind it (subtile dep tracking)
                    for h4 in range(4):
                        nc.vector.tensor_copy(
                            out=V_aug[:, h4, kb, 0:64],
                            in_=vp[:, h4 * 64:(h4 + 1) * 64])

        def v_halves(kb):
            st = {}
            return [lambda: emit_v(kb, st, 0), lambda: emit_v(kb, st, 1)]

        def emit_outproj(qb, dc, last=False):
            outp = ps_p.tile([128, 512], f32, tag="qk", name="outp")
            for p in range(2):
                nc.tensor.matmul(
                    out=outp,
                    lhsT=OT[:, p, qb * 128:(qb + 1) * 128],
                    rhs=Wo_sb[:, p, dc * 512:(dc + 1) * 512],
                    start=(p == 0), stop=(p == 1))
            ob = pstg.tile([128, 512], f32, tag="ob", name="ob")
            if last:
                # tail: spread drain copies over ACT + DVE, DMAs over rings
                if (qb + dc) % 2:
                    nc.scalar.copy(out=ob, in_=outp)
                    dma = nc.gpsimd.dma_start
                else:
                    nc.vector.tensor_copy(out=ob, in_=outp)
                    dma = nc.sync.dma_start
                dma(out=out_d[qb * 128:(qb + 1) * 128,
                              dc * 512:(dc + 1) * 512],
                    in_=ob)
            else:
                nc.vector.tensor_copy(out=ob, in_=outp)
                nc.sync.dma_start(
                    out=out_d[qb * 128:(qb + 1) * 128,
                              dc * 512:(dc + 1) * 512],
                    in_=ob)

        # ---- software-pipelined O: S/exp run LAG key-blocks ahead of the
        # O accumulation, so projection fillers get LAG grps of deadline
        # slack and the PE never stalls on an exp it just requested ----
        LAG = 6
        ostate = {"q": [], "grp": 0, "ops": {}}

        def finalize(p, qc):
            ops = ostate["ops"].pop((p, qc))
            for hh in range(2):
                # early release: stage denominator + numerator out of PSUM,
                # then normalize off the critical path
                den = pv.tile([1, 512], f32, tag="den", name="den")
                nc.vector.tensor_copy(out=den, in_=ops[hh][64:65, :])
                onum = pv.tile([64, 512], f32, tag="onum", name="onum")
                nc.vector.tensor_copy(out=onum, in_=ops[hh][0:64, :])
                recip = pv.tile([1, 512], f32, tag="recip", name="recip")
                nc.vector.reciprocal_approx_fast(out=recip, in_=den)
                rbs = pv.tile([64, 512], f32, tag="rbs", name="rbs")
                nc.gpsimd.partition_broadcast(rbs, recip)
                nc.vector.tensor_mul(
                    OT[hh * 64:(hh + 1) * 64, p, qc * 512:(qc + 1) * 512],
                    onum, rbs)

        def drain_one():
            p, qc, kb, es = ostate["q"].pop(0)
            if kb == 0:
                ostate["ops"][(p, qc)] = [
                    psO.tile([128, 512], f32, tag="o", name=f"op{hh}")
                    for hh in range(2)]
            ops = ostate["ops"][(p, qc)]
            for hh in range(2):
                nc.tensor.matmul(
                    out=ops[hh],
                    lhsT=V_aug[:, p * 2 + hh, kb, :],
                    rhs=es[:, hh, :],
                    start=(kb == 0), stop=(kb == KB - 1))
            if kb == KB - 1:
                finalize(p, qc)

        def attn_qc(p, qc, st, last=False):
            for kb in range(KB):
                npop = st[0].pop(0) if st[0] else 0
                for _ in range(npop):
                    if st[1]:
                        st[1].pop(0)()
                sp = psS.tile([128, 2, 512], f32, tag="s", name="sp")
                for hh in range(2):
                    nc.tensor.matmul(
                        out=sp[:, hh, :],
                        lhsT=KT[hh * 64:(hh + 1) * 64, p,
                                kb * 128:(kb + 1) * 128],
                        rhs=QT_nat[hh * 64:(hh + 1) * 64, p,
                                   qc * 512:(qc + 1) * 512],
                        start=True, stop=True)
                es = pes.tile([128, 2, 512], bf16, tag="es", name="es")
                nc.scalar.activation(out=es, in_=sp, func=Exp, scale=SCALE)
                ostate["q"].append((p, qc, kb, es))
                ostate["grp"] += 1
                if ostate["grp"] > LAG:
                    drain_one()
                if last and len(ostate["q"]) > 3 and ostate["grp"] > LAG:
                    drain_one()  # shrink the pipeline lag toward the end
                    # (not to zero: O(kb) right after exp(kb) would stall
                    # the in-order PE on the exp it just requested)

        # ---------------- emission order (= PE stream order) -------------
        emit_qt(0, 0)
        emit_kt(0, 0)
        emit_qt(0, 1)
        emit_v(0)
        emit_v(1)

        OP = lambda qb, dc: (lambda: emit_outproj(qb, dc))

        # deadline-checked filler schedule (LAG=6 deferral): V(kb) must pop
        # by grp kb+LAG (where its first O matmul drains); K^T chunk kc by
        # the S matmul of grp 4*kc (pops precede the S matmuls of the same
        # grp); pair-1 projections by the end of (0,3).  V bundles split in
        # 4-contraction-step halves (0.44us) so early grps stay under the
        # 1.15us exp pace.
        sched = {
            (0, 0): ([2, 2, 2, 2, 2, 2] + [1] * 10,
                     proj_halves(emit_kt, 0, 1)
                     + v_halves(2) + v_halves(3) + v_halves(4) + v_halves(5)
                     + proj_halves(emit_kt, 0, 2)
                     + v_halves(6)
                     + proj_halves(emit_kt, 0, 3)
                     + v_halves(7) + v_halves(8) + v_halves(9)),
            (0, 1): ([2, 2, 2, 2, 2, 2] + [1] * 4 + [0] * 6,
                     v_halves(10) + v_halves(11) + v_halves(12)
                     + v_halves(13) + v_halves(14) + v_halves(15)
                     + proj_halves(emit_qt, 0, 2)
                     + proj_halves(emit_qt, 0, 3)),
            (0, 2): ([1] * 10 + [0] * 6,
                     proj_halves(emit_qt, 1, 0)
                     + proj_halves(emit_kt, 1, 0)
                     + proj_halves(emit_qt, 1, 1)
                     + proj_halves(emit_kt, 1, 1)
                     + proj_halves(emit_qt, 1, 2)),
            (0, 3): ([1] * 6 + [0] * 10,
                     proj_halves(emit_kt, 1, 2)
                     + proj_halves(emit_qt, 1, 3)
                     + proj_halves(emit_kt, 1, 3)),
            (1, 0): ([0] * 16, []),
            (1, 1): ([0] * 8 + [1] * 8,
                     [OP(qb, dc) for qb in range(0, 4) for dc in range(2)]),
            (1, 2): ([0] * 8 + [1] * 8,
                     [OP(qb, dc) for qb in range(4, 8) for dc in range(2)]),
            (1, 3): ([0] * 6 + [1] * 8 + [0] * 2,
                     [OP(qb, dc) for qb in range(8, 12) for dc in range(2)]),
        }
        carry = []
        for p in range(2):
            for qc in range(QC):
                pops, fills = sched[(p, qc)]
                st = [list(pops), carry + fills]
                attn_qc(p, qc, st, last=(p == 1 and qc == 3))
                carry = st[1]
        for f in carry:
            f()
        while ostate["q"]:
            drain_one()
        for qb in range(12, 16):
            for dc in range(2):
                emit_outproj(qb, dc, last=True)

    nc.compile()
    return nc


def _get_nc():
    if "nc" not in _cached:
        _cached["nc"] = _build()
    return _cached["nc"]


def kernel(queries, Wq, Wkv, Wo, bo, _trace=False):
    import ml_dtypes
    from concourse.bass_utils import run_bass_kernel_spmd

    bf = ml_dtypes.bfloat16
    queries = np.asarray(queries, dtype=np.float32)
    Wq = np.asarray(Wq, dtype=np.float32)
    Wkv = np.asarray(Wkv, dtype=np.float32)
    Wo = np.asarray(Wo, dtype=np.float32)
    bo = np.asarray(bo, dtype=np.float32)

    nc = _get_nc()

    xT_g = [np.ascontiguousarray(queries[g].T).astype(bf) for g in range(B)]
    in_maps = []
    for c in range(NCORES):
        g, hq = c // 4, c % 4
        s, e = hq * HS, (hq + 1) * HS
        in_maps.append({
            "xT": xT_g[g],
            "Wq": np.ascontiguousarray(Wq[:, s:e]).astype(bf),
            "Wk": np.ascontiguousarray(Wkv[:, s:e]).astype(bf),
            "Wv": np.ascontiguousarray(Wkv[:, INNER + s:INNER + e]).astype(bf),
            "Wo": np.ascontiguousarray(Wo[s:e, :]).astype(bf),
        })

    res = run_bass_kernel_spmd(nc, in_maps, list(range(NCORES)),
                               trace=_trace)
    out = np.empty((B, N, DM), dtype=np.float32)
    for g in range(B):
        acc = res.results[4 * g]["out"].astype(np.float32)
        for r in range(1, 4):
            acc = acc + res.results[4 * g + r]["out"]
        out[g] = acc + bo[None, :]
    if _trace:
        return out, res
    return out


if __name__ == "__main__":
    rng = np.random.default_rng(0)
    s = 0.02
    inputs = dict(
        queries=rng.standard_normal((B, N, DM), dtype=np.float32),
        Wq=(rng.standard_normal((DM, INNER), dtype=np.float32) * s),
        Wkv=(rng.standard_normal((DM, 2 * INNER), dtype=np.float32) * s),
        Wo=(rng.standard_normal((INNER, DM), dtype=np.float32) * s),
        bo=(rng.standard_normal((DM,), dtype=np.float32) * s),
    )
    out = kernel(**inputs)
    print("kernel ran, out shape", out.shape)

    q = inputs["queries"]
    qp = q @ inputs["Wq"]
    kv = q @ inputs["Wkv"]
    k, v = np.split(kv, 2, axis=-1)
    sh = lambda t: t.reshape(B, N, H, DH).transpose(0, 2, 1, 3)
    qp, k, v = map(sh, (qp, k, v))
    sim = np.einsum('bhid,bhjd->bhij', qp, k) * SCALE
    sim = np.exp(sim - sim.max(-1, keepdims=True))
    attn = sim / sim.sum(-1, keepdims=True)
    o = np.einsum('bhij,bhjd->bhid', attn, v)
    o = o.transpose(0, 2, 1, 3).reshape(B, N, INNER)
    exp = o @ inputs["Wo"] + inputs["bo"]
    err = np.linalg.norm((out - exp).ravel()) / np.linalg.norm(exp.ravel())
    print("numpy rel err:", err)


# revision 33
# speedup vs baseline: 1.0488x; 1.0209x over previous
"""Trainium2 Bass kernel for 16-head self-attention (b=2, n=2048, dm=1024, dh=64).

Sharding: (batch x head-quad).  Core c owns batch g = c//4 and heads
[4*(c%4) .. 4*(c%4)+3], a 256-column slice of the inner dimension.  Every
projection is computed exactly once across the chip (393k PE cycles/core vs
590k for the replicated batch x seq sharding of v1).  Each core's output
projection is a PARTIAL sum over its 256 inner dims; the host sums the four
partials per batch during the unshard (the "all-reduce after to_out" of the
sharding hint, folded into the host gather).

All matmuls are bf16 (separate LDWEIGHTS instructions hide behind in-flight
matmuls via the PE reorder window; f32r self-loads weights serially and
measured +34us/core).  Everything is SBUF-resident.

S^T is ROW-TILED: each head contracts only dh=64, so the two heads of a
pair run CONCURRENTLY in disjoint 64-row strips of the PE array
(tile_position auto-derived from base partition 0 / 64) -- one ~512-cycle
pass produces S^T for both heads, halving S matmul time vs the v3
zero-padded-contraction scheme and making Q^T zero-padding unnecessary.

Layouts (no on-chip transposes):
  Q^T[i,q] = (Wq slice as lhsT) @ (x^T as rhs)
  K^T[i,k] = (Wk slice as lhsT) @ (x^T as rhs)
  V [k,i]  = (x^T as lhsT) @ (Wv slice as rhs)  stored as [V_h | 1 | 0pad]
             so PSUM row 64 of the O matmul is the softmax denominator
  S^T[k,q] = (K^T rows of head h as lhsT) @ (Q^T rows of head h as rhs),
             both heads concurrent in row strips -> [128, 2, 512] PSUM
  O''[d,q] = ([V|1|0] as lhsT) @ (exp S^T as rhs)  accumulated over 16 kb
  out[q,d] = (O^T as lhsT) @ (Wo slice as rhs)   partial; host sums

exp runs on ACT over [128,1024] two-bank PSUM groups (one per key block,
both heads) -- 128 instructions at (1024+352)/1.2 ns = 143us/core, the
attention pacing engine.  The PE stream interleaves projection /
output-projection work into the ACT-bound idle via a deadline-scheduled
filler list (projection bundles split into 4-contraction-step halves so a
single pop never stalls the exp pipe by more than ~0.9us).  PSUM: S 2x2 +
O 2 + proj 2 = 8 banks.

Softmax finalize is early-release: the denominator row and the 64
numerator rows are staged out of PSUM with two DVE copies (releasing the
O PSUM bank for the next query chunk immediately); reciprocal_approx_fast
(on the SBUF copy -- the custom DVE op misreads PSUM operands) + GPSIMD
partition_broadcast + multiply then run off the critical path.
"""

import sys

for _p in ("/opt/trn_rl_repo", "/root/.axon_site/_ro/trn_rl_repo"):
    if _p not in sys.path:
        sys.path.append(_p)

import numpy as np

B = 2
N = 2048
DM = 1024
H = 16
DH = 64
INNER = H * DH  # 1024
NCORES = 8
HS = 256        # inner slice per core (4 heads)
SCALE = DH ** -0.5

A = DM // 128   # 8 dm blocks
KB = N // 128   # 16 key blocks
QC = N // 512   # 4 query chunks

_cached = {}


def _build():
    import contextlib
    import concourse.bacc as bacc
    import concourse.tile as tile
    import concourse.mybir as mybir

    f32 = mybir.dt.float32
    bf16 = mybir.dt.bfloat16
    Exp = mybir.ActivationFunctionType.Exp

    nc = bacc.Bacc("TRN2", target_bir_lowering=False, debug=False,
                   enable_asserts=False)

    xT_d = nc.dram_tensor("xT", [DM, N], bf16, kind="ExternalInput").ap()
    Wq_d = nc.dram_tensor("Wq", [DM, HS], bf16, kind="ExternalInput").ap()
    Wk_d = nc.dram_tensor("Wk", [DM, HS], bf16, kind="ExternalInput").ap()
    Wv_d = nc.dram_tensor("Wv", [DM, HS], bf16, kind="ExternalInput").ap()
    Wo_d = nc.dram_tensor("Wo", [HS, DM], bf16, kind="ExternalInput").ap()
    out_d = nc.dram_tensor("out", [N, DM], f32, kind="ExternalOutput").ap()

    with tile.TileContext(nc) as tc, \
         nc.allow_low_precision(reason="bf16 matmul pipeline, validated "
                                       "e2e vs f32 reference"), \
         contextlib.ExitStack() as ctx:
        persist = ctx.enter_context(tc.tile_pool(name="persist", bufs=1))
        QT_nat = persist.tile([128, 2, N], bf16)       # [pair dims, pair, q]
        KT = persist.tile([128, 2, N], bf16)           # [pair dims, pair, k]
        V_aug = persist.tile([128, 4, KB, 128], bf16)  # [keys, head, kb, V|1|0]
        OT = persist.tile([128, 2, N], bf16)           # [pair dims, pair, q]
        Wo_sb = persist.tile([128, 2, DM], bf16)
        ozpat = persist.tile([128, 64], f32)           # col0=1, cols1..63=0

        pa_x = ctx.enter_context(tc.tile_pool(name="pa_x", bufs=1))
        pa_w = ctx.enter_context(tc.tile_pool(name="pa_w", bufs=1))
        pes = ctx.enter_context(tc.tile_pool(name="pes", bufs=8))
        pv = ctx.enter_context(tc.tile_pool(name="pv", bufs=2))
        pstg = ctx.enter_context(tc.tile_pool(name="pstg", bufs=4))
        ps_p = ctx.enter_context(
            tc.tile_pool(name="ps_p", bufs=2, space="PSUM"))
        psS = ctx.enter_context(
            tc.tile_pool(name="psS", bufs=2, space="PSUM"))
        psO = ctx.enter_context(
            tc.tile_pool(name="psO", bufs=2, space="PSUM"))

        xT_sb = pa_x.tile([128, A, N], bf16)
        Wq_sb = pa_w.tile([128, A, HS], bf16)
        Wk_sb = pa_w.tile([128, A, HS], bf16)
        Wv_sb = pa_w.tile([128, A, HS], bf16)

        # constant pads: DVE broadcast fill during the DMA-gated dead time
        nc.vector.memset(ozpat, 0.0)
        nc.vector.memset(ozpat[:, 0:1], 1.0)
        nc.vector.tensor_copy(
            out=V_aug[:, :, :, 64:128],
            in_=ozpat.unsqueeze(1).unsqueeze(1).to_broadcast(
                [128, 4, KB, 64]))

        xT_r = xT_d.rearrange("(a p) n -> a p n", p=128)
        # sync-ring DMA order: what the PE needs first, first
        nc.sync.dma_start(out=Wq_sb,
                          in_=Wq_d.rearrange("(a p) i -> p a i", p=128))
        for a in range(A):
            nc.sync.dma_start(out=xT_sb[:, a, 0:512], in_=xT_r[a][:, 0:512])
        nc.sync.dma_start(out=Wk_sb,
                          in_=Wk_d.rearrange("(a p) i -> p a i", p=128))
        nc.sync.dma_start(out=Wv_sb,
                          in_=Wv_d.rearrange("(a p) i -> p a i", p=128))
        for a in range(A):
            nc.sync.dma_start(out=xT_sb[:, a, 512:1024],
                              in_=xT_r[a][:, 512:1024])
        for qc in range(2, QC):
            for a in range(A):
                nc.sync.dma_start(out=xT_sb[:, a, qc * 512:(qc + 1) * 512],
                                  in_=xT_r[a][:, qc * 512:(qc + 1) * 512])
        nc.sync.dma_start(out=Wo_sb,
                          in_=Wo_d.rearrange("(ib p) d -> p ib d", p=128))

        # ---- emission helpers ----
        def emit_qt(p, qc, state=None, half=None):
            if state is None:
                state = {}
                halves = (0, 1)
            else:
                halves = (half,)
            for hf in halves:
                if hf == 0:
                    state['t'] = ps_p.tile([128, 512], f32, tag="qk",
                                           name="qp")
                qp = state['t']
                for a in range(hf * 4, hf * 4 + 4):
                    nc.tensor.matmul(
                        out=qp,
                        lhsT=Wq_sb[:, a, p * 128:(p + 1) * 128],
                        rhs=xT_sb[:, a, qc * 512:(qc + 1) * 512],
                        start=(a == 0), stop=(a == A - 1))
                if hf == 1:
                    nc.vector.tensor_copy(
                        out=QT_nat[:, p, qc * 512:(qc + 1) * 512], in_=qp)

        def emit_kt(p, kc, state=None, half=None):
            if state is None:
                state = {}
                halves = (0, 1)
            else:
                halves = (half,)
            for hf in halves:
                if hf == 0:
                    state['t'] = ps_p.tile([128, 512], f32, tag="qk",
                                           name="kp")
                kp = state['t']
                for a in range(hf * 4, hf * 4 + 4):
                    nc.tensor.matmul(
                        out=kp,
                        lhsT=Wk_sb[:, a, p * 128:(p + 1) * 128],
                        rhs=xT_sb[:, a, kc * 512:(kc + 1) * 512],
                        start=(a == 0), stop=(a == A - 1))
                if hf == 1:
                    nc.vector.tensor_copy(
                        out=KT[:, p, kc * 512:(kc + 1) * 512], in_=kp)

        def proj_halves(fn, p, i):
            st = {}
            return [lambda: fn(p, i, st, 0), lambda: fn(p, i, st, 1)]

        def emit_v(kb):
            vp = ps_p.tile([128, HS], f32, tag="qk", name="vp")
            for a in range(A):
                nc.tensor.matmul(
                    out=vp,
                    lhsT=xT_sb[:, a, kb * 128:(kb + 1) * 128],
                    rhs=Wv_sb[:, a, :],
                    start=(a == 0), stop=(a == A - 1))
            nc.vector.tensor_copy(
                out=V_aug[:, :, kb, 0:64],
                in_=vp.rearrange("p (h d) -> p h d", h=4))

        def emit_outproj(qb, dc, last=False):
            outp = ps_p.tile([128, 512], f32, tag="qk", name="outp")
            for p in range(2):
                nc.tensor.matmul(
                    out=outp,
                    lhsT=OT[:, p, qb * 128:(qb + 1) * 128],
                    rhs=Wo_sb[:, p, dc * 512:(dc + 1) * 512],
                    start=(p == 0), stop=(p == 1))
            ob = pstg.tile([128, 512], f32, tag="ob", name="ob")
            if last:
                nc.scalar.copy(out=ob, in_=outp)  # ACT is idle at the tail
                dma = (nc.gpsimd.dma_start if (qb + dc) % 2
                       else nc.sync.dma_start)
                dma(out=out_d[qb * 128:(qb + 1) * 128,
                              dc * 512:(dc + 1) * 512],
                    in_=ob)
            else:
                nc.vector.tensor_copy(out=ob, in_=outp)
                nc.sync.dma_start(
                    out=out_d[qb * 128:(qb + 1) * 128,
                              dc * 512:(dc + 1) * 512],
                    in_=ob)

        # ---- software-pipelined O: S/exp run LAG key-blocks ahead of the
        # O accumulation, so projection fillers get LAG grps of deadline
        # slack and the PE never stalls on an exp it just requested ----
        LAG = 6
        ostate = {"q": [], "grp": 0, "ops": {}}

        def finalize(p, qc):
            ops = ostate["ops"].pop((p, qc))
            for hh in range(2):
                # early release: stage denominator + numerator out of PSUM,
                # then normalize off the critical path
                den = pv.tile([1, 512], f32, tag="den", name="den")
                nc.vector.tensor_copy(out=den, in_=ops[hh][64:65, :])
                onum = pv.tile([64, 512], f32, tag="onum", name="onum")
                nc.vector.tensor_copy(out=onum, in_=ops[hh][0:64, :])
                recip = pv.tile([1, 512], f32, tag="recip", name="recip")
                nc.vector.reciprocal_approx_fast(out=recip, in_=den)
                rbs = pv.tile([64, 512], f32, tag="rbs", name="rbs")
                nc.gpsimd.partition_broadcast(rbs, recip)
                nc.vector.tensor_mul(
                    OT[hh * 64:(hh + 1) * 64, p, qc * 512:(qc + 1) * 512],
                    onum, rbs)

        def drain_one():
            p, qc, kb, es = ostate["q"].pop(0)
            if kb == 0:
                ostate["ops"][(p, qc)] = [
                    psO.tile([128, 512], f32, tag="o", name=f"op{hh}")
                    for hh in range(2)]
            ops = ostate["ops"][(p, qc)]
            for hh in range(2):
                nc.tensor.matmul(
                    out=ops[hh],
                    lhsT=V_aug[:, p * 2 + hh, kb, :],
                    rhs=es[:, hh, :],
                    start=(kb == 0), stop=(kb == KB - 1))
            if kb == KB - 1:
                finalize(p, qc)

        def attn_qc(p, qc, st):
            for kb in range(KB):
                npop = st[0].pop(0) if st[0] else 0
                for _ in range(npop):
                    if st[1]:
                        st[1].pop(0)()
                sp = psS.tile([128, 2, 512], f32, tag="s", name="sp")
                for hh in range(2):
                    nc.tensor.matmul(
                        out=sp[:, hh, :],
                        lhsT=KT[hh * 64:(hh + 1) * 64, p,
                                kb * 128:(kb + 1) * 128],
                        rhs=QT_nat[hh * 64:(hh + 1) * 64, p,
                                   qc * 512:(qc + 1) * 512],
                        start=True, stop=True)
                es = pes.tile([128, 2, 512], bf16, tag="es", name="es")
                nc.scalar.activation(out=es, in_=sp, func=Exp, scale=SCALE)
                ostate["q"].append((p, qc, kb, es))
                ostate["grp"] += 1
                if ostate["grp"] > LAG:
                    drain_one()

        # ---------------- emission order (= PE stream order) -------------
        emit_qt(0, 0)
        emit_kt(0, 0)
        emit_qt(0, 1)

        V = lambda kb: (lambda: emit_v(kb))
        OP = lambda qb, dc: (lambda: emit_outproj(qb, dc))

        # deadline-checked filler schedule (LAG=6 deferral): V(kb) must pop
        # by grp kb+LAG (where its first O matmul drains); K^T chunk kc by
        # the S matmul of grp 4*kc (pops precede the S matmuls of the same
        # grp); pair-1 projections by the end of (0,3).
        sched = {
            (0, 0): ([1] * 16,
                     proj_halves(emit_kt, 0, 1) + [V(0), V(1), V(2), V(3)]
                     + proj_halves(emit_kt, 0, 2) + [V(4), V(5)]
                     + proj_halves(emit_kt, 0, 3)
                     + [V(6), V(7), V(8), V(9)]),
            (0, 1): ([1] * 10 + [0] * 6,
                     [V(10), V(11), V(12), V(13), V(14), V(15)]
                     + proj_halves(emit_qt, 0, 2)
                     + proj_halves(emit_qt, 0, 3)),
            (0, 2): ([1] * 4 + [0] * 12,
                     proj_halves(emit_qt, 1, 0)
                     + proj_halves(emit_kt, 1, 0)),
            (0, 3): ([0] * 16, []),
            # pair-1 K^T chunks kc1-3 and Q^T qc1-3 pop inside (1,0) itself
            # (K^T chunk kc is only read from S grp 4*kc on; Q^T qc from
            # (1,qc)) -- pair-0 was filler-overloaded, pair-1 has slack
            (1, 0): ([1] * 12 + [0] * 4,
                     proj_halves(emit_kt, 1, 1)
                     + proj_halves(emit_kt, 1, 2)
                     + proj_halves(emit_kt, 1, 3)
                     + proj_halves(emit_qt, 1, 1)
                     + proj_halves(emit_qt, 1, 2)
                     + proj_halves(emit_qt, 1, 3)),
            (1, 1): ([0] * 8 + [1] * 8,
                     [OP(qb, dc) for qb in range(0, 4) for dc in range(2)]),
            (1, 2): ([0] * 8 + [1] * 8,
                     [OP(qb, dc) for qb in range(4, 8) for dc in range(2)]),
            (1, 3): ([0] * 8 + [1] * 8,
                     [OP(qb, dc) for qb in range(8, 12) for dc in range(2)]),
        }
        carry = []
        for p in range(2):
            for qc in range(QC):
                pops, fills = sched[(p, qc)]
                st = [list(pops), carry + fills]
                attn_qc(p, qc, st)
                carry = st[1]
        for f in carry:
            f()
        while ostate["q"]:
            drain_one()
        for qb in range(12, 16):
            for dc in range(2):
                emit_outproj(qb, dc, last=True)

    nc.compile()
    return nc


def _get_nc():
    if "nc" not in _cached:
        _cached["nc"] = _build()
    return _cached["nc"]


def kernel(queries, Wq, Wkv, Wo, bo, _trace=False):
    import ml_dtypes
    from concourse.bass_utils import run_bass_kernel_spmd

    bf = ml_dtypes.bfloat16
    queries = np.asarray(queries, dtype=np.float32)
    Wq = np.asarray(Wq, dtype=np.float32)
    Wkv = np.asarray(Wkv, dtype=np.float32)
    Wo = np.asarray(Wo, dtype=np.float32)
    bo = np.asarray(bo, dtype=np.float32)

    nc = _get_nc()

    xT_g = [np.ascontiguousarray(queries[g].T).astype(bf) for g in range(B)]
    in_maps = []
    for c in range(NCORES):
        g, hq = c // 4, c % 4
        s, e = hq * HS, (hq + 1) * HS
        in_maps.append({
            "xT": xT_g[g],
            "Wq": np.ascontiguousarray(Wq[:, s:e]).astype(bf),
            "Wk": np.ascontiguousarray(Wkv[:, s:e]).astype(bf),
            "Wv": np.ascontiguousarray(Wkv[:, INNER + s:INNER + e]).astype(bf),
            "Wo": np.ascontiguousarray(Wo[s:e, :]).astype(bf),
        })

    res = run_bass_kernel_spmd(nc, in_maps, list(range(NCORES)),
                               trace=_trace)
    out = np.empty((B, N, DM), dtype=np.float32)
    for g in range(B):
        acc = res.results[4 * g]["out"].astype(np.float32)
        for r in range(1, 4):
            acc = acc + res.results[4 * g + r]["out"]
        out[g] = acc + bo[None, :]
    if _trace:
        return out, res
    return out


if __name__ == "__main__":
    rng = np.random.default_rng(0)
    s = 0.02
    inputs = dict(
        queries=rng.standard_normal((B, N, DM), dtype=np.float32),
        Wq=(rng.standard_normal((DM, INNER), dtype=np.float32) * s),
        Wkv=(rng.standard_normal((DM, 2 * INNER), dtype=np.float32) * s),
        Wo=(rng.standard_normal((INNER, DM), dtype=np.float32) * s),
        bo=(rng.standard_normal((DM,), dtype=np.float32) * s),
    )
    out = kernel(**inputs)
    print("kernel ran, out shape", out.shape)

    q = inputs["queries"]
    qp = q @ inputs["Wq"]
    kv = q @ inputs["Wkv"]
    k, v = np.split(kv, 2, axis=-1)
    sh = lambda t: t.reshape(B, N, H, DH).transpose(0, 2, 1, 3)
    qp, k, v = map(sh, (qp, k, v))
    sim = np.einsum('bhid,bhjd->bhij', qp, k) * SCALE
    sim = np.exp(sim - sim.max(-1, keepdims=True))
    attn = sim / sim.sum(-1, keepdims=True)
    o = np.einsum('bhij,bhjd->bhid', attn, v)
    o = o.transpose(0, 2, 1, 3).reshape(B, N, INNER)
    exp = o @ inputs["Wo"] + inputs["bo"]
    err = np.linalg.norm((out - exp).ravel()) / np.linalg.norm(exp.ravel())
    print("numpy rel err:", err)
